# revision 16
# baseline (speedup 1.0000x reference)
"""Trainium2 Bass kernel for EquivariantGraphConvCheap (gnn_message_passing).

Strategy (8 cores, no collectives):
  - Destination-node sharding: core c owns dest nodes [c*6250, (c+1)*6250).
  - Host packs edges by (core, dest-block of 128, src-half); device gathers
    source rows x[col] (fp16, 1KB each) via dma_gather, segment-sums them
    with a per-chunk 0/1 selection matrix on the TensorEngine into PSUM
    (agg[dest, feat]), transposes per channel, then applies the rel/root
    128x128 weight matmuls + bias, writing out[node, 512] directly.
  - int16 gather indices => source array split at row 25000 (lo/hi halves).

Everything numeric on device is fp16 with fp32 PSUM accumulation.
"""
import os
import numpy as np

import concourse.bacc as bacc
import concourse.mybir as mybir
import concourse.tile as tile
from concourse import bass_utils
from concourse.masks import make_identity

# ---- hardcoded problem geometry ----
N = 50000
E = 500000
H = 128
D = 4 * H            # 512 features per node row
NCORES = 8
NPC = N // NCORES    # 6250 dest nodes per core
NB = -(-NPC // 128)  # 49 dest blocks per core (last partial)
SPLIT = 25000        # source split for int16 indices
CAPL_MIN = 6         # chunks of 128 edges per block, low half
CAPH_MIN = 6

f16 = mybir.dt.float16
f32 = mybir.dt.float32
i16 = mybir.dt.int16


def _hoist_extra_waits(nc, max_waits=1):
    """Walrus codegen allows a limited number of sync waits per instruction.
    Hoist all but the last wait onto InstNoOp carriers inserted immediately
    before the offending instruction (same engine => same dispatch stream)."""
    n_fixed = 0
    for fn in nc.m.functions:
        for blk in fn.blocks:
            new_insts = []
            for ins in blk.instructions:
                si = ins.sync_info
                if si is not None and si.on_wait and len(si.on_wait) > max_waits:
                    waits = list(si.on_wait)
                    for j, w in enumerate(waits[:-max_waits]):
                        nop = mybir.InstNoOp(
                            name=f"{ins.name}-waitnop{j}", ins=[], outs=[])
                        nop.engine = ins.engine
                        nop.sync_info = mybir.SyncInfo(on_wait=[w], on_update=[])
                        new_insts.append(nop)
                    ins.sync_info = mybir.SyncInfo(
                        on_wait=waits[-max_waits:],
                        on_update=list(si.on_update or []))
                    n_fixed += 1
                new_insts.append(ins)
            blk.instructions[:] = new_insts
    return n_fixed


def build_nc(nb, capl, caph, n_lo, n_hi, hoist=True):
    """Build the per-core Bass program (SPMD: same NEFF on all cores)."""
    ch_tot = capl + caph
    iw = nb * ch_tot * 8          # idx cols (int16, wrapped by 16)
    soff = 128                    # cst: slots start after iota
    woff = soff + nb * ch_tot     # cst: weights start after slots
    cw = woff + 8 * 128           # cst total cols

    nc = bacc.Bacc("TRN2", target_bir_lowering=False, debug=False,
                   num_swdge_queues=4)
    x_lo = nc.dram_tensor("x_lo", (n_lo, D), f16, kind="ExternalInput")
    x_hi = nc.dram_tensor("x_hi", (n_hi, D), f16, kind="ExternalInput")
    idx_d = nc.dram_tensor("idx", (128, iw), i16, kind="ExternalInput")
    cnt_d = nc.dram_tensor("cnt", (1, 2 * nb), mybir.dt.int32, kind="ExternalInput")
    cst_d = nc.dram_tensor("cst", (128, cw), f16, kind="ExternalInput")
    bias_d = nc.dram_tensor("bias", (1, 128), f16, kind="ExternalInput")
    xt_d = nc.dram_tensor("xt", (nb, 128, D), f16, kind="ExternalInput")
    out_d = nc.dram_tensor("out", (nb * 128, D), f32, kind="ExternalOutput")

    with tile.TileContext(nc) as tc:
        with tc.tile_pool(name="const", bufs=1) as cp, \
             tc.tile_pool(name="gather", bufs=3) as gp, \
             tc.tile_pool(name="sel", bufs=3) as sp, \
             tc.tile_pool(name="aggps", bufs=2, space="PSUM") as aps, \
             tc.tile_pool(name="aggsb", bufs=2) as asb, \
             tc.tile_pool(name="trps", bufs=2, space="PSUM") as tps, \
             tc.tile_pool(name="aggT", bufs=2) as atp, \
             tc.tile_pool(name="xtp", bufs=3) as xtp, \
             tc.tile_pool(name="outps", bufs=2, space="PSUM") as ops_, \
             tc.tile_pool(name="outsb", bufs=3) as osb:

            cnt_sb = cp.tile([1, 2 * nb], mybir.dt.int32)
            nc.sync.dma_start(out=cnt_sb[:], in_=cnt_d.ap())
            cst_sb = cp.tile([128, cw], f16)
            nc.sync.dma_start(out=cst_sb[:], in_=cst_d.ap())
            idx_sb = cp.tile([128, iw], i16)
            nc.sync.dma_start(out=idx_sb[:], in_=idx_d.ap())
            bias_sb = cp.tile([1, 128], f16)
            nc.sync.dma_start(out=bias_sb[:], in_=bias_d.ap())
            ones_sb = cp.tile([1, 128], f16)
            nc.vector.memset(ones_sb[:], 1.0)
            ident = cp.tile([128, 128], f16)
            make_identity(nc, ident[:])

            iota_b = cst_sb[:, 0:128][:, None, :]

            # persistent rotating gather tiles. Blocks 0-2 gather at full
            # static capacity (zero-padded indices) so every byte of each
            # tile is written before any matmul reads it; later blocks use
            # runtime counts (negative-tail skip) and may leave stale -- but
            # finite -- fp16 from 3 blocks ago in the padding region, which
            # the zero rows of S nullify.
            t_tiles = []
            for _i in range(3):
                _tt = gp.tile([128, ch_tot, D], f16, tag=f"t_tile{_i}", name=f"t_tile{_i}")
                t_tiles.append(_tt)

            import contextlib
            _regctx = contextlib.ExitStack()
            lo_reg = _regctx.enter_context(nc.gpsimd.register("lo_cnt"))
            hi_reg = _regctx.enter_context(nc.gpsimd.register("hi_cnt"))

            for b in range(nb):
                # ---- gather source rows for this dest block ----
                t_tile = t_tiles[b % 3]
                i0 = b * ch_tot * 8
                if b >= 3:
                    nc.gpsimd.reg_load(lo_reg, cnt_sb[0:1, 2 * b:2 * b + 1])
                    lo_cnt = lo_reg
                else:
                    lo_cnt = capl * 128
                nc.gpsimd.dma_gather(
                    out_ap=t_tile[:, 0:capl, :], in_ap=x_lo.ap(),
                    idxs_ap=idx_sb[:, i0:i0 + capl * 8],
                    num_idxs=capl * 128, num_idxs_reg=lo_cnt,
                    elem_size=D, queue_num=b % 4, single_packet=False)
                if b >= 3:
                    nc.gpsimd.reg_load(hi_reg, cnt_sb[0:1, 2 * b + 1:2 * b + 2])
                    hi_cnt = hi_reg
                else:
                    hi_cnt = caph * 128
                nc.gpsimd.dma_gather(
                    out_ap=t_tile[:, capl:ch_tot, :], in_ap=x_hi.ap(),
                    idxs_ap=idx_sb[:, i0 + capl * 8:i0 + ch_tot * 8],
                    num_idxs=caph * 128, num_idxs_reg=hi_cnt,
                    elem_size=D, queue_num=(b + 2) % 4, single_packet=False)

                # ---- selection matrices S[edge, dest_slot] for the block ----
                s_tile = sp.tile([128, ch_tot, 128], f16)
                nc.vector.tensor_tensor(
                    out=s_tile[:],
                    in0=iota_b.to_broadcast([128, ch_tot, 128]),
                    in1=cst_sb[:, soff + b * ch_tot:soff + (b + 1) * ch_tot]
                        [:, :, None].to_broadcast([128, ch_tot, 128]),
                    op=mybir.AluOpType.is_equal)

                # ---- segment sum: agg[dest, feat] += S^T @ T ----
                agg_ps = aps.tile([128, D], f32, space="PSUM")
                for cc in range(ch_tot):
                    nc.tensor.matmul(
                        out=agg_ps[:], lhsT=s_tile[:, cc, :], rhs=t_tile[:, cc, :],
                        start=(cc == 0), stop=(cc == ch_tot - 1))
                agg_sb = asb.tile([128, D], f16)
                nc.scalar.copy(out=agg_sb[:], in_=agg_ps[:])

                # ---- transpose each channel: aggT[h, dest] ----
                tr_ps = tps.tile([128, D], f16, space="PSUM")
                for ch in range(4):
                    nc.tensor.transpose(
                        out=tr_ps[:, ch * 128:(ch + 1) * 128],
                        in_=agg_sb[:, ch * 128:(ch + 1) * 128],
                        identity=ident[:])
                aggt_sb = atp.tile([128, D], f16)
                nc.vector.tensor_copy(out=aggt_sb[:], in_=tr_ps[:])

                # ---- own-node features (transposed) ----
                xt_sb = xtp.tile([128, D], f16)
                nc.sync.dma_start(out=xt_sb[:], in_=xt_d.ap()[b])

                # ---- output: out[node, o] = agg @ Wrel^T + own @ Wroot^T (+ b) ----
                out_ps = ops_.tile([128, D], f32, space="PSUM")
                for ch in range(4):
                    reg = out_ps[:, ch * 128:(ch + 1) * 128]
                    nc.tensor.matmul(
                        out=reg, lhsT=aggt_sb[:, ch * 128:(ch + 1) * 128],
                        rhs=cst_sb[:, woff + ch * 128:woff + (ch + 1) * 128],
                        start=True, stop=False)
                    nc.tensor.matmul(
                        out=reg, lhsT=xt_sb[:, ch * 128:(ch + 1) * 128],
                        rhs=cst_sb[:, woff + 512 + ch * 128:woff + 512 + (ch + 1) * 128],
                        start=False, stop=(ch != 0))
                    if ch == 0:
                        nc.tensor.matmul(
                            out=reg, lhsT=ones_sb[:], rhs=bias_sb[:],
                            start=False, stop=True)
                out_sb = osb.tile([128, D], f32)
                nc.scalar.copy(out=out_sb[:], in_=out_ps[:])
                nc.sync.dma_start(
                    out=out_d.ap()[b * 128:(b + 1) * 128, :], in_=out_sb[:])
            _regctx.close()

    nc.compile()
    if hoist:
        _hoist_extra_waits(nc)
    return nc


def _wrap_idx(vals, cap, pad):
    """int16 values (<= cap*128) -> [128, cap*8] wrapped-by-16 layout."""
    vp = np.full(cap * 128, pad, dtype=np.int16)
    vp[:len(vals)] = vals
    w16 = vp.reshape(cap * 8, 16).T          # [16, cap*8]
    return np.tile(w16, (8, 1))              # [128, cap*8]


def pack_inputs(x, edge_index, W_s_rel, W_s_root, b_s_root, W_v_rel, W_v_root,
                n=N, ncores=NCORES, capl_min=CAPL_MIN, caph_min=CAPH_MIN,
                split=None):
    """Host-side packing. Returns (in_maps, meta)."""
    npc = n // ncores
    nb = -(-npc // 128)
    if split is None:
        split = (n // 2 + 127) & ~127 if n != N else SPLIT
    x = np.asarray(x)
    xr = np.ascontiguousarray(x.reshape(n, D)).astype(np.float16)
    row = np.asarray(edge_index[0]).astype(np.int64)
    col = np.asarray(edge_index[1]).astype(np.int64)

    core = row // npc
    rloc = row % npc
    block = rloc // 128
    slot = rloc % 128
    half = (col >= split).astype(np.int64)
    gkey = (core * nb + block) * 2 + half
    order = np.argsort(gkey, kind="stable")
    gk_s = gkey[order]
    col_s = col[order]
    slot_s = slot[order]
    counts = np.bincount(gk_s, minlength=ncores * nb * 2)
    starts = np.concatenate([[0], np.cumsum(counts)])

    capl = max(capl_min, int(-(-counts[0::2].max() // 128)))
    caph = max(caph_min, int(-(-counts[1::2].max() // 128)))
    ch_tot = capl + caph

    # weights in fp16, transposed layout W_T[h, o] = W[o, h]
    rels = [W_s_rel, W_v_rel, W_v_rel, W_v_rel]
    roots = [W_s_root, W_v_root, W_v_root, W_v_root]
    soff = 128
    woff = soff + nb * ch_tot
    cw = woff + 8 * 128
    cst_common = np.zeros((128, cw), dtype=np.float16)
    cst_common[:, 0:128] = np.arange(128, dtype=np.float16)[None, :]
    for ch in range(4):
        cst_common[:, woff + ch * 128:woff + (ch + 1) * 128] = \
            np.asarray(rels[ch]).T.astype(np.float16)
        cst_common[:, woff + 512 + ch * 128:woff + 512 + (ch + 1) * 128] = \
            np.asarray(roots[ch]).T.astype(np.float16)
    bias = np.asarray(b_s_root).astype(np.float16).reshape(1, 128)

    x_lo = xr[:split]
    x_hi = xr[split:]

    in_maps = []
    for c in range(ncores):
        idx_arr = np.zeros((128, nb * ch_tot * 8), dtype=np.int16)
        cnt_arr = np.zeros((1, 2 * nb), dtype=np.int32)
        cst = cst_common.copy()
        for b in range(nb):
            for hh, cap, base in ((0, capl, 0), (1, caph, capl)):
                g = (c * nb + b) * 2 + hh
                s0, s1 = starts[g], starts[g + 1]
                ncnt = s1 - s0
                cnt_arr[0, 2 * b + hh] = ncnt
                assert ncnt <= cap * 128, (
                    f"block overflow: core {c} block {b} half {hh}: {ncnt}")
                vals = col_s[s0:s1] - (split if hh else 0)
                i0 = b * ch_tot * 8 + base * 8
                idx_arr[:, i0:i0 + cap * 8] = _wrap_idx(
                    vals.astype(np.int16), cap, 0 if b < 3 else -1)
                sp_ = np.full(cap * 128, -1.0, dtype=np.float16)
                sp_[:ncnt] = slot_s[s0:s1].astype(np.float16)
                cst[:, soff + b * ch_tot + base:soff + b * ch_tot + base + cap] = \
                    sp_.reshape(cap, 128).T
        # own nodes, transposed per block: xt[b, h, ch*128+n]
        xpad = np.zeros((nb * 128, 4, H), dtype=np.float16)
        xpad[:npc] = x[c * npc:(c + 1) * npc].astype(np.float16)
        xt = np.ascontiguousarray(
            xpad.reshape(nb, 128, 4, H).transpose(0, 3, 2, 1).reshape(nb, 128, D))
        in_maps.append({
            "x_lo": x_lo, "x_hi": x_hi, "idx": idx_arr, "cst": cst,
            "bias": bias, "xt": xt, "cnt": cnt_arr,
        })
    meta = dict(nb=nb, capl=capl, caph=caph, n_lo=split, n_hi=n - split,
                npc=npc, ncores=ncores)
    return in_maps, meta


_NC_CACHE = {}
LAST_RESULTS = None


def run(x, edge_index, W_s_rel, W_s_root, b_s_root, W_v_rel, W_v_root,
        n=N, ncores=NCORES, trace=False):
    global LAST_RESULTS
    in_maps, meta = pack_inputs(
        x, edge_index, W_s_rel, W_s_root, b_s_root, W_v_rel, W_v_root,
        n=n, ncores=ncores)
    key = (meta["nb"], meta["capl"], meta["caph"], meta["n_lo"], meta["n_hi"])
    if key not in _NC_CACHE:
        _NC_CACHE[key] = build_nc(*key)
    nc = _NC_CACHE[key]
    res = bass_utils.run_bass_kernel_spmd(
        nc, in_maps, core_ids=list(range(ncores)), trace=trace)
    LAST_RESULTS = res
    npc = meta["npc"]
    parts = [res.results[c]["out"][:npc] for c in range(ncores)]
    out = np.concatenate(parts, axis=0).reshape(n, 4, H).astype(np.float32)
    return out


def kernel(x, edge_index, W_s_rel, W_s_root, b_s_root, W_v_rel, W_v_root):
    return run(x, edge_index, W_s_rel, W_s_root, b_s_root, W_v_rel, W_v_root,
               trace=bool(os.environ.get("BASS_TRACE")))


# revision 17
# speedup vs baseline: 1.0707x; 1.0707x over previous
"""Trainium2 Bass kernel for EquivariantGraphConvCheap (gnn_message_passing).

Strategy (8 cores, no collectives):
  - Destination-node sharding: core c owns dest nodes [c*6250, (c+1)*6250).
  - Host packs edges by (core, dest-block of 128, src-half); device gathers
    source rows x[col] (fp16, 1KB each) via dma_gather, segment-sums them
    with a per-chunk 0/1 selection matrix on the TensorEngine into PSUM
    (agg[dest, feat]), transposes per channel, then applies the rel/root
    128x128 weight matmuls + bias, writing out[node, 512] directly.
  - int16 gather indices => source array split at row 25000 (lo/hi halves).

Everything numeric on device is fp16 with fp32 PSUM accumulation.
"""
import os
import numpy as np

import concourse.bacc as bacc
import concourse.mybir as mybir
import concourse.tile as tile
from concourse import bass_utils
from concourse.masks import make_identity

# ---- hardcoded problem geometry ----
N = 50000
E = 500000
H = 128
D = 4 * H            # 512 features per node row
NCORES = 8
NPC = N // NCORES    # 6250 dest nodes per core
NB = -(-NPC // 128)  # 49 dest blocks per core (last partial)
SPLIT = 25000        # source split for int16 indices
CAPL_MIN = 6         # chunks of 128 edges per block, low half
CAPH_MIN = 6

f16 = mybir.dt.float16
f32 = mybir.dt.float32
i16 = mybir.dt.int16


def _hoist_extra_waits(nc, max_waits=1):
    """Walrus codegen allows a limited number of sync waits per instruction.
    Hoist all but the last wait onto InstNoOp carriers inserted immediately
    before the offending instruction (same engine => same dispatch stream)."""
    n_fixed = 0
    for fn in nc.m.functions:
        for blk in fn.blocks:
            new_insts = []
            for ins in blk.instructions:
                si = ins.sync_info
                if si is not None and si.on_wait and len(si.on_wait) > max_waits:
                    waits = list(si.on_wait)
                    for j, w in enumerate(waits[:-max_waits]):
                        nop = mybir.InstNoOp(
                            name=f"{ins.name}-waitnop{j}", ins=[], outs=[])
                        nop.engine = ins.engine
                        nop.sync_info = mybir.SyncInfo(on_wait=[w], on_update=[])
                        new_insts.append(nop)
                    ins.sync_info = mybir.SyncInfo(
                        on_wait=waits[-max_waits:],
                        on_update=list(si.on_update or []))
                    n_fixed += 1
                new_insts.append(ins)
            blk.instructions[:] = new_insts
    return n_fixed


def build_nc(nb, capl, caph, n_lo, n_hi, hoist=True):
    """Build the per-core Bass program (SPMD: same NEFF on all cores)."""
    ch_tot = capl + caph
    iw = nb * ch_tot * 8          # idx cols (int16, wrapped by 16)
    soff = 128                    # cst: slots start after iota
    woff = soff + nb * ch_tot     # cst: weights start after slots
    cw = woff + 8 * 128           # cst total cols

    nc = bacc.Bacc("TRN2", target_bir_lowering=False, debug=False,
                   num_swdge_queues=4)
    x_lo = nc.dram_tensor("x_lo", (n_lo, D), f16, kind="ExternalInput")
    x_hi = nc.dram_tensor("x_hi", (n_hi, D), f16, kind="ExternalInput")
    idx_d = nc.dram_tensor("idx", (128, iw), i16, kind="ExternalInput")
    cnt_d = nc.dram_tensor("cnt", (1, 2 * nb), mybir.dt.int32, kind="ExternalInput")
    cst_d = nc.dram_tensor("cst", (128, cw), f16, kind="ExternalInput")
    bias_d = nc.dram_tensor("bias", (1, 128), f16, kind="ExternalInput")
    xt_d = nc.dram_tensor("xt", (nb, 128, D), f16, kind="ExternalInput")
    out_d = nc.dram_tensor("out", (nb * 128, D), f32, kind="ExternalOutput")

    with tile.TileContext(nc) as tc:
        with tc.tile_pool(name="const", bufs=1) as cp, \
             tc.tile_pool(name="gather", bufs=3) as gp, \
             tc.tile_pool(name="sel", bufs=3) as sp, \
             tc.tile_pool(name="aggps", bufs=2, space="PSUM") as aps, \
             tc.tile_pool(name="aggsb", bufs=2) as asb, \
             tc.tile_pool(name="trps", bufs=2, space="PSUM") as tps, \
             tc.tile_pool(name="aggT", bufs=2) as atp, \
             tc.tile_pool(name="xtp", bufs=3) as xtp, \
             tc.tile_pool(name="outps", bufs=2, space="PSUM") as ops_, \
             tc.tile_pool(name="outsb", bufs=3) as osb:

            cnt_sb = cp.tile([1, 2 * nb], mybir.dt.int32)
            nc.sync.dma_start(out=cnt_sb[:], in_=cnt_d.ap())
            cst_sb = cp.tile([128, cw], f16)
            nc.sync.dma_start(out=cst_sb[:], in_=cst_d.ap())
            idx_sb = cp.tile([128, iw], i16)
            nc.sync.dma_start(out=idx_sb[:], in_=idx_d.ap())
            bias_sb = cp.tile([1, 128], f16)
            nc.sync.dma_start(out=bias_sb[:], in_=bias_d.ap())
            ones_sb = cp.tile([1, 128], f16)
            nc.vector.memset(ones_sb[:], 1.0)
            ident = cp.tile([128, 128], f16)
            make_identity(nc, ident[:])

            iota_b = cst_sb[:, 0:128][:, None, :]

            # persistent rotating gather tiles. Blocks 0-2 gather at full
            # static capacity (zero-padded indices) so every byte of each
            # tile is written before any matmul reads it; later blocks use
            # runtime counts (negative-tail skip) and may leave stale -- but
            # finite -- fp16 from 3 blocks ago in the padding region, which
            # the zero rows of S nullify.
            t_tiles = []
            for _i in range(3):
                _tt = gp.tile([128, ch_tot, D], f16, tag=f"t_tile{_i}", name=f"t_tile{_i}")
                t_tiles.append(_tt)

            import contextlib
            _regctx = contextlib.ExitStack()
            lo_reg = _regctx.enter_context(nc.gpsimd.register("lo_cnt"))
            hi_reg = _regctx.enter_context(nc.gpsimd.register("hi_cnt"))

            for b in range(nb):
                # ---- gather source rows for this dest block ----
                t_tile = t_tiles[b % 3]
                i0 = b * ch_tot * 8
                if b >= 3:
                    nc.gpsimd.reg_load(lo_reg, cnt_sb[0:1, 2 * b:2 * b + 1])
                    lo_cnt = lo_reg
                else:
                    lo_cnt = capl * 128
                nc.gpsimd.dma_gather(
                    out_ap=t_tile[:, 0:capl, :], in_ap=x_lo.ap(),
                    idxs_ap=idx_sb[:, i0:i0 + capl * 8],
                    num_idxs=capl * 128, num_idxs_reg=lo_cnt,
                    elem_size=D, queue_num=(b % 2) * 2, single_packet=False)
                if b >= 3:
                    nc.gpsimd.reg_load(hi_reg, cnt_sb[0:1, 2 * b + 1:2 * b + 2])
                    hi_cnt = hi_reg
                else:
                    hi_cnt = caph * 128
                nc.gpsimd.dma_gather(
                    out_ap=t_tile[:, capl:ch_tot, :], in_ap=x_hi.ap(),
                    idxs_ap=idx_sb[:, i0 + capl * 8:i0 + ch_tot * 8],
                    num_idxs=caph * 128, num_idxs_reg=hi_cnt,
                    elem_size=D, queue_num=(b % 2) * 2 + 1, single_packet=False)

                # ---- selection matrices S[edge, dest_slot] for the block ----
                s_tile = sp.tile([128, ch_tot, 128], f16)
                nc.vector.tensor_tensor(
                    out=s_tile[:],
                    in0=iota_b.to_broadcast([128, ch_tot, 128]),
                    in1=cst_sb[:, soff + b * ch_tot:soff + (b + 1) * ch_tot]
                        [:, :, None].to_broadcast([128, ch_tot, 128]),
                    op=mybir.AluOpType.is_equal)

                # ---- segment sum: agg[dest, feat] += S^T @ T ----
                agg_ps = aps.tile([128, D], f32, space="PSUM")
                for cc in range(ch_tot):
                    nc.tensor.matmul(
                        out=agg_ps[:], lhsT=s_tile[:, cc, :], rhs=t_tile[:, cc, :],
                        start=(cc == 0), stop=(cc == ch_tot - 1))
                agg_sb = asb.tile([128, D], f16)
                nc.scalar.copy(out=agg_sb[:], in_=agg_ps[:])

                # ---- transpose each channel: aggT[h, dest] ----
                tr_ps = tps.tile([128, D], f16, space="PSUM")
                for ch in range(4):
                    nc.tensor.transpose(
                        out=tr_ps[:, ch * 128:(ch + 1) * 128],
                        in_=agg_sb[:, ch * 128:(ch + 1) * 128],
                        identity=ident[:])
                aggt_sb = atp.tile([128, D], f16)
                nc.vector.tensor_copy(out=aggt_sb[:], in_=tr_ps[:])

                # ---- own-node features (transposed) ----
                xt_sb = xtp.tile([128, D], f16)
                nc.sync.dma_start(out=xt_sb[:], in_=xt_d.ap()[b])

                # ---- output: out[node, o] = agg @ Wrel^T + own @ Wroot^T (+ b) ----
                out_ps = ops_.tile([128, D], f32, space="PSUM")
                for ch in range(4):
                    reg = out_ps[:, ch * 128:(ch + 1) * 128]
                    nc.tensor.matmul(
                        out=reg, lhsT=aggt_sb[:, ch * 128:(ch + 1) * 128],
                        rhs=cst_sb[:, woff + ch * 128:woff + (ch + 1) * 128],
                        start=True, stop=False)
                    nc.tensor.matmul(
                        out=reg, lhsT=xt_sb[:, ch * 128:(ch + 1) * 128],
                        rhs=cst_sb[:, woff + 512 + ch * 128:woff + 512 + (ch + 1) * 128],
                        start=False, stop=(ch != 0))
                    if ch == 0:
                        nc.tensor.matmul(
                            out=reg, lhsT=ones_sb[:], rhs=bias_sb[:],
                            start=False, stop=True)
                out_sb = osb.tile([128, D], f32)
                nc.scalar.copy(out=out_sb[:], in_=out_ps[:])
                nc.sync.dma_start(
                    out=out_d.ap()[b * 128:(b + 1) * 128, :], in_=out_sb[:])
            _regctx.close()

    nc.compile()
    if hoist:
        _hoist_extra_waits(nc)
    return nc


def _wrap_idx(vals, cap, pad):
    """int16 values (<= cap*128) -> [128, cap*8] wrapped-by-16 layout."""
    vp = np.full(cap * 128, pad, dtype=np.int16)
    vp[:len(vals)] = vals
    w16 = vp.reshape(cap * 8, 16).T          # [16, cap*8]
    return np.tile(w16, (8, 1))              # [128, cap*8]


def pack_inputs(x, edge_index, W_s_rel, W_s_root, b_s_root, W_v_rel, W_v_root,
                n=N, ncores=NCORES, capl_min=CAPL_MIN, caph_min=CAPH_MIN,
                split=None):
    """Host-side packing. Returns (in_maps, meta)."""
    npc = n // ncores
    nb = -(-npc // 128)
    if split is None:
        split = (n // 2 + 127) & ~127 if n != N else SPLIT
    x = np.asarray(x)
    xr = np.ascontiguousarray(x.reshape(n, D)).astype(np.float16)
    row = np.asarray(edge_index[0]).astype(np.int64)
    col = np.asarray(edge_index[1]).astype(np.int64)

    core = row // npc
    rloc = row % npc
    block = rloc // 128
    slot = rloc % 128
    half = (col >= split).astype(np.int64)
    gkey = (core * nb + block) * 2 + half
    order = np.argsort(gkey, kind="stable")
    gk_s = gkey[order]
    col_s = col[order]
    slot_s = slot[order]
    counts = np.bincount(gk_s, minlength=ncores * nb * 2)
    starts = np.concatenate([[0], np.cumsum(counts)])

    capl = max(capl_min, int(-(-counts[0::2].max() // 128)))
    caph = max(caph_min, int(-(-counts[1::2].max() // 128)))
    ch_tot = capl + caph

    # weights in fp16, transposed layout W_T[h, o] = W[o, h]
    rels = [W_s_rel, W_v_rel, W_v_rel, W_v_rel]
    roots = [W_s_root, W_v_root, W_v_root, W_v_root]
    soff = 128
    woff = soff + nb * ch_tot
    cw = woff + 8 * 128
    cst_common = np.zeros((128, cw), dtype=np.float16)
    cst_common[:, 0:128] = np.arange(128, dtype=np.float16)[None, :]
    for ch in range(4):
        cst_common[:, woff + ch * 128:woff + (ch + 1) * 128] = \
            np.asarray(rels[ch]).T.astype(np.float16)
        cst_common[:, woff + 512 + ch * 128:woff + 512 + (ch + 1) * 128] = \
            np.asarray(roots[ch]).T.astype(np.float16)
    bias = np.asarray(b_s_root).astype(np.float16).reshape(1, 128)

    x_lo = xr[:split]
    x_hi = xr[split:]

    in_maps = []
    for c in range(ncores):
        idx_arr = np.zeros((128, nb * ch_tot * 8), dtype=np.int16)
        cnt_arr = np.zeros((1, 2 * nb), dtype=np.int32)
        cst = cst_common.copy()
        for b in range(nb):
            for hh, cap, base in ((0, capl, 0), (1, caph, capl)):
                g = (c * nb + b) * 2 + hh
                s0, s1 = starts[g], starts[g + 1]
                ncnt = s1 - s0
                cnt_arr[0, 2 * b + hh] = ncnt
                assert ncnt <= cap * 128, (
                    f"block overflow: core {c} block {b} half {hh}: {ncnt}")
                vals = col_s[s0:s1] - (split if hh else 0)
                i0 = b * ch_tot * 8 + base * 8
                idx_arr[:, i0:i0 + cap * 8] = _wrap_idx(
                    vals.astype(np.int16), cap, 0 if b < 3 else -1)
                sp_ = np.full(cap * 128, -1.0, dtype=np.float16)
                sp_[:ncnt] = slot_s[s0:s1].astype(np.float16)
                cst[:, soff + b * ch_tot + base:soff + b * ch_tot + base + cap] = \
                    sp_.reshape(cap, 128).T
        # own nodes, transposed per block: xt[b, h, ch*128+n]
        xpad = np.zeros((nb * 128, 4, H), dtype=np.float16)
        xpad[:npc] = x[c * npc:(c + 1) * npc].astype(np.float16)
        xt = np.ascontiguousarray(
            xpad.reshape(nb, 128, 4, H).transpose(0, 3, 2, 1).reshape(nb, 128, D))
        in_maps.append({
            "x_lo": x_lo, "x_hi": x_hi, "idx": idx_arr, "cst": cst,
            "bias": bias, "xt": xt, "cnt": cnt_arr,
        })
    meta = dict(nb=nb, capl=capl, caph=caph, n_lo=split, n_hi=n - split,
                npc=npc, ncores=ncores)
    return in_maps, meta


_NC_CACHE = {}
LAST_RESULTS = None


def run(x, edge_index, W_s_rel, W_s_root, b_s_root, W_v_rel, W_v_root,
        n=N, ncores=NCORES, trace=False):
    global LAST_RESULTS
    in_maps, meta = pack_inputs(
        x, edge_index, W_s_rel, W_s_root, b_s_root, W_v_rel, W_v_root,
        n=n, ncores=ncores)
    key = (meta["nb"], meta["capl"], meta["caph"], meta["n_lo"], meta["n_hi"])
    if key not in _NC_CACHE:
        _NC_CACHE[key] = build_nc(*key)
    nc = _NC_CACHE[key]
    res = bass_utils.run_bass_kernel_spmd(
        nc, in_maps, core_ids=list(range(ncores)), trace=trace)
    LAST_RESULTS = res
    npc = meta["npc"]
    parts = [res.results[c]["out"][:npc] for c in range(ncores)]
    out = np.concatenate(parts, axis=0).reshape(n, 4, H).astype(np.float32)
    return out


def kernel(x, edge_index, W_s_rel, W_s_root, b_s_root, W_v_rel, W_v_root):
    return run(x, edge_index, W_s_rel, W_s_root, b_s_root, W_v_rel, W_v_root,
               trace=bool(os.environ.get("BASS_TRACE")))
